# revision 41
# baseline (speedup 1.0000x reference)
"""Masked-softmax attention pooling on 8 TRN2 NeuronCores — fp16 streaming.

Reference computation (per batch b):
    q = hidden @ W.T                      # [H]
    alphas[s] = eo[b, s, :] . q           # [S]
    alphas = where(mask, -1e16, alphas)
    scores = softmax(alphas)              # over S
    out[b] = sum_s scores[s] * eo[b, s, :]

Sharding: data-parallel over batch (8 batches/core), W replicated.

encoder_output dominates traffic; it is host-cast to fp16 (validated
rel_l2 ~2e-3 vs the f32 reference on this instance) and streamed from
HBM exactly once at half the f32 byte count: 4 chunks per batch, each
[128, 4*H] fp16 = 1 MiB per dma_start with 8 KiB contiguous partition
lines (s = c*512 + 4p + t), alternating between the two HWDGE rings.
The landed fp16 tile feeds BOTH the alphas dot products and the
TensorE weighted sum directly — no recast copy.

The alphas dot products are the elementwise wall (every eo element
needs one multiply + one add into a row-sum).  DVE's fused
scalar_tensor_tensor runs at 1x (1217 ns/[128,1024] col), so the 16
columns are split: 6 run the fused STT on DVE; 10 run the multiply on
DVE tensor_tensor at 2x fp16 (594 ns) and hand the f32 row-sum to
ScalarE (Copy-activation with accum_out) — balancing DVE and ScalarE.
(GPSIMD is NOT used for elementwise work: it shares SBUF ports with
DVE and running both slows DVE ~1.7x — measured.)

q path: host supplies W^T (row-permuted for 16 KiB partition lines)
and hidden^T in fp16; q is 16 plain matmuls with zero on-device
transposes.  q[b] is broadcast to 128 partitions via a selector-matrix
matmul for batch 0 (fast pipeline start) and via a DRAM-bounce
replicating DMA for b>=1 (zero engine cost, prefetched a batch ahead).

Softmax is the two-level scheme: per-partition max/sum on the [128,16]
alphas tile, then a cross-partition fix-up via three tiny TensorE
transposes; 1/den folds into the per-partition factor g, and
us = e*g is written fp16 to feed the weighted-sum matmuls.

Weighted sum: per column, stationary us[:,col] against the moving eo
tile, col-tiled with tile_position=(0, 32*(b%4)) so four batches
accumulate into disjoint partition rows of one [128,512]x2 PSUM pair;
one [128,512] ScalarE copy per 4 batches replaces per-batch [1,1024]
single-lane copies.  Small keepalive matmuls on fresh alphas columns
keep the PE HAM clock at 2.4 GHz between weighted-sum bursts.

Measured on 8 axon-tunneled TRN2 cores: 196-198 us (f32 baseline of the
same algorithm: 254 us; the original staged kernel: 301 us quoted).
Steady state is bound by DVE+ScalarE elementwise throughput (~16
us/batch each), with the eo stream (~12.5 us/batch) hidden beneath it.
"""

from contextlib import ExitStack

import numpy as np

import concourse.bass as bass
import concourse.tile as tile
from concourse import bacc, mybir
from concourse._compat import get_trn_type
from concourse.bass_utils import run_bass_kernel_spmd
from concourse.masks import make_identity

B, S, H = 64, 2048, 1024
N_CORES = 8
BL = B // N_CORES      # 8 batches per core
RP = 4                 # s-rows interleaved per partition (8 KiB fp16 lines)
NCH = S // (128 * RP)  # 4 DMA chunks per batch, 512 rows each
SC = S // 128          # 16 alphas columns (col = chunk*RP + t)
HC = H // 128
F32 = mybir.dt.float32
F16 = mybir.dt.float16

NEG_BIG = -1.0e16
EO_BUFS = 18           # fp16 landing tiles [128, RP*H] (8 KiB/partition each)

# per-chunk column plan: "stt" = DVE fused scalar_tensor_tensor;
# everything else = DVE tensor_tensor (2x fp16) + ScalarE Copy-accum sum
STT_T = {0: (0, 2), 1: (1, 3), 2: (0, 2), 3: ()}


def _build(BL=BL, S=S, H=H):
    nc = bacc.Bacc(get_trn_type() or "TRN2", target_bir_lowering=False)

    eo_d = nc.dram_tensor("encoder_output", [BL, S, H], F16, kind="ExternalInput")
    # host-prepped W^T with rows [p, c]-major (wt[p*HC + c, :] = W.T[c*128+p, :])
    wt_d = nc.dram_tensor("W", [H, H], F16, kind="ExternalInput")
    # hidden^T in partition-major layout ht[p, c*BL + b] = hidden[b, c*128 + p]
    ht_d = nc.dram_tensor("hidden", [128, HC * BL], F16, kind="ExternalInput")
    # host-prepped additive mask: [128, BL, SC], -1e16 at masked positions,
    # column index col = chunk*RP + t  <->  s = chunk*512 + 4p + t
    mk_d = nc.dram_tensor("encoder_mask", [128, BL, SC], F32, kind="ExternalInput")
    # host-prepped broadcast selector: sel[p, i] = (p == 0)
    sel_d = nc.dram_tensor("sel", [BL, 128], F16, kind="ExternalInput")
    out_d = nc.dram_tensor("out", [BL, H], F32, kind="ExternalOutput")

    AF = mybir.ActivationFunctionType
    ALU = mybir.AluOpType
    AX = mybir.AxisListType

    with tile.TileContext(nc) as tc, ExitStack() as ctx:
        sing = ctx.enter_context(tc.tile_pool(name="sing", bufs=1))
        eop = ctx.enter_context(tc.tile_pool(name="eop", bufs=EO_BUFS))
        trashp = ctx.enter_context(tc.tile_pool(name="trashp", bufs=1))
        smallp = ctx.enter_context(tc.tile_pool(name="smallp", bufs=2))
        qbp = ctx.enter_context(tc.tile_pool(name="qbp", bufs=2))
        outp = ctx.enter_context(tc.tile_pool(name="outp", bufs=2))
        ps_tr = ctx.enter_context(tc.tile_pool(name="ps_tr", bufs=2, space="PSUM"))
        ps_qb = ctx.enter_context(tc.tile_pool(name="ps_qb", bufs=1, space="PSUM"))
        ps_c = ctx.enter_context(tc.tile_pool(name="ps_c", bufs=4, space="PSUM"))
        dramp = ctx.enter_context(tc.tile_pool(name="dramp", bufs=1, space="DRAM"))

        ident = sing.tile([128, 128], F32)
        make_identity(nc, ident[:])

        # warm the PE HAM clock while the first DMAs land: ~40 tiny matmuls
        # keep the PE busy through the first SHORT window so the q matmuls
        # and the first weighted sums run at 2.4 GHz, not 1.2
        warm_ps = ps_tr.tile([1, 128], F32, tag="tr")
        for w in range(40):
            nc.tensor.matmul(
                warm_ps[0:1, :], ident[:, 0:1], ident[:, :],
                start=True, stop=True,
            )

        # ---- W^T rides the SP ring AHEAD of the eo stream (two 1 MiB DMAs
        # with 16 KiB partition lines); hT/sel lead the ACT ring so batch
        # 0's odd chunks follow immediately; the mask (first needed by
        # finish_batch(0)) queues behind them
        wt = sing.tile([128, HC, H], F16)   # W^T, [128h, c, 1024o]
        wt_v = wt_d.rearrange("(p c) h -> p (c h)", p=128)
        nc.sync.dma_start(wt.rearrange("p c h -> p (c h)")[:, : 4 * H], wt_v[:, : 4 * H])
        nc.sync.dma_start(wt.rearrange("p c h -> p (c h)")[:, 4 * H :], wt_v[:, 4 * H :])
        hT = sing.tile([128, HC, BL], F16)  # hidden^T, [128h, c, BL]
        nc.scalar.dma_start(hT.rearrange("p c b -> p (c b)"), ht_d[:])
        mask_sb = sing.tile([128, BL, SC], F32)
        nc.scalar.dma_start(mask_sb[:], mk_d[:])
        sel = sing.tile([BL, 128], F16)
        nc.scalar.dma_start(sel[:], sel_d[:])

        # ---- q = hidden @ W.T via pre-transposed operands: 16 matmuls,
        # each nh-half accumulating over the 8 h-chunks
        q_ps = ps_qb.tile([128, H], F32, tag="qb")
        for nh in range(H // 512):
            for c in range(HC):
                nc.tensor.matmul(
                    q_ps[0:BL, bass.ts(nh, 512)],
                    hT[:, c, :],
                    wt[:, c, bass.ts(nh, 512)],
                    start=(c == 0),
                    stop=(c == HC - 1),
                )
        q_sb = sing.tile([BL, H], F16)
        nc.scalar.copy(q_sb[:], q_ps[0:BL, :])
        q_dram = dramp.tile([BL, H], F16)
        nc.gpsimd.dma_start(q_dram[:], q_sb[:])  # SWDGE: keeps the HWDGE rings clear

        # ---- main loop over local batches, software-pipelined: batch b's
        # softmax + weighted sum are emitted BETWEEN batch b+1's chunks so
        # no engine queue head-of-line blocks on the softmax chain
        state = {}
        dmas = {}

        def start_batch(b):
            qb = qbp.tile([128, H], F16, tag="qb16")
            if b == 0:
                # selector matmul broadcast (sel[p, i] = (p == 0)): fast
                # pipeline start that avoids the DRAM round trip
                qb_ps = ps_qb.tile([128, H], F32, tag="qb")
                for nh in range(H // 512):
                    nc.tensor.matmul(
                        qb_ps[:, bass.ts(nh, 512)],
                        sel[:, :],
                        q_sb[:, bass.ts(nh, 512)],
                        start=True,
                        stop=True,
                    )
                nc.scalar.copy(qb[:], qb_ps[:])
            else:
                nc.gpsimd.dma_start(
                    qb[:], q_dram[b : b + 1, :].to_broadcast([128, H])
                )
            alphas = smallp.tile([128, SC], F32, tag="alphas")
            state[b] = (qb, alphas)

        def emit_chunk(b, c):
            qb, alphas = state[b]
            # [NCH, 128, RP*H] view: 8 KiB contiguous per partition line;
            # chunks alternate between the two HWDGE rings
            eo_v = eo_d[b].rearrange("(c p t) h -> c p (t h)", p=128, t=RP)
            x = eop.tile([128, RP, H], F16, tag="x", name=f"x_{b}_{c}")
            ring = nc.sync if c % 2 == 0 else nc.scalar
            ring.dma_start(x.rearrange("p t h -> p (t h)"), eo_v[c])
            dmas.setdefault(b, {})[c] = x
            for t in range(RP):
                col = c * RP + t
                acol = alphas[:, col : col + 1]
                if t in STT_T[c]:
                    trash = trashp.tile([128, H], F16, tag="trashv")
                    nc.vector.scalar_tensor_tensor(
                        out=trash[:], in0=x[:, t, :], scalar=1.0, in1=qb[:],
                        op0=ALU.mult, op1=ALU.mult,
                        accum_out=acol,
                    )
                else:
                    prod = trashp.tile([128, H], F16, tag="prodv", bufs=4)
                    nc.vector.tensor_mul(prod[:], x[:, t, :], qb[:])
                    junk = trashp.tile([128, H], F16, tag="junk", bufs=2)
                    nc.scalar.activation(
                        out=junk[:], in_=prod[:], func=AF.Copy,
                        bias=0.0, scale=1.0, accum_out=acol,
                    )
                # keepalive: a tiny matmul on the freshly written alphas
                # column, spread across the batch period so the PE HAM
                # never sees a >3.4us idle window and the weighted-sum
                # matmuls stay at 2.4 GHz
                ka = ps_tr.tile([1, 1], F32, tag="tr")
                nc.tensor.matmul(
                    ka[:], acol, acol,
                    start=True, stop=True,
                )

        cgrp = {}  # (gen, nh) -> PSUM tile accumulating 4 batches

        def finish_batch(b):
            qb, alphas = state.pop(b)
            xs = dmas.pop(b)
            am = smallp.tile([128, SC], F32, tag="am")
            nc.vector.tensor_add(am[:], alphas[:], mask_sb[:, b, :])

            # two-level softmax. Row-level (per partition p over its 16 cols):
            #   nm[p] = -max_col am[p,col]
            #   e[p,col] = exp(am[p,col] + nm[p]);  s1[p] = sum_col e[p,col]
            # Cross-partition fix-up:
            #   mn = min_p nm[p]  (= -global max)
            #   g[p] = exp(-(nm[p] - mn)) / den;  den = sum_p s1[p]*exp(-(nm-mn))
            #   us[p,col] = e[p,col] * g[p]  (fp16, feeds TensorE directly)
            m1 = smallp.tile([128, 1], F32, tag="m1")
            nc.vector.tensor_reduce(
                out=m1[:], in_=am[:], axis=AX.X, op=ALU.max,
            )
            nm = smallp.tile([128, 1], F32, tag="nm")
            nc.scalar.mul(nm[:], m1[:], -1.0)
            e = smallp.tile([128, SC], F32, tag="e")
            s1 = smallp.tile([128, 1], F32, tag="s1")
            nc.scalar.activation(
                out=e[:], in_=am[:], func=AF.Exp,
                bias=nm[:], scale=1.0, accum_out=s1[:],
            )
            nmp = ps_tr.tile([1, 128], F32, tag="tr")
            nc.tensor.transpose(nmp[:], nm[:], ident[:])
            s1p = ps_tr.tile([1, 128], F32, tag="tr")
            nc.tensor.transpose(s1p[:], s1[:], ident[:])
            mn = smallp.tile([1, 1], F32, tag="mn")
            nc.vector.tensor_reduce(out=mn[:], in_=nmp[:], axis=AX.X, op=ALU.min)
            dn = smallp.tile([1, 128], F32, tag="dn")
            nc.vector.tensor_scalar_sub(dn[:], nmp[:], mn[0:1, 0:1])
            g = smallp.tile([1, 128], F32, tag="g")
            nc.scalar.activation(out=g[:], in_=dn[:], func=AF.Exp, bias=0.0, scale=-1.0)
            wtr = smallp.tile([1, 128], F32, tag="wtr")
            nc.vector.tensor_mul(wtr[:], s1p[:], g[:])
            den = smallp.tile([1, 1], F32, tag="den")
            nc.vector.tensor_reduce(
                out=den[:], in_=wtr[:], axis=AX.X, op=ALU.add,
            )
            r = smallp.tile([1, 1], F32, tag="r")
            nc.vector.reciprocal(r[:], den[:])
            gr = smallp.tile([1, 128], F32, tag="gr")
            nc.vector.tensor_scalar_mul(gr[:], g[:], r[0:1, 0:1])
            gp = ps_tr.tile([128, 1], F32, tag="tr")
            nc.tensor.transpose(gp[:], gr[:], ident[0:1, 0:1])
            us = smallp.tile([128, SC], F16, tag="us")
            nc.vector.tensor_scalar_mul(us[:], e[:], gp[:, 0:1])

            # c = sum_s us[s] * eo[s, :]  via TensorE in fp16, score column
            # stationary, eo moving; col-tiled so 4 batches accumulate into
            # disjoint partition rows (32*(b%4)) of one [128,512]x2 pair
            gen, row = divmod(b, 4)
            if row == 0:
                for nh in range(H // 512):
                    cgrp[(gen, nh)] = ps_c.tile([128, 512], F32, tag="c", name=f"cacc_{gen}_{nh}")
            for c in range(NCH):
                for t in range(RP):
                    col = c * RP + t
                    for nh in range(H // 512):
                        nc.tensor.matmul(
                            cgrp[(gen, nh)][32 * row : 32 * row + 1, :],
                            us[:, col : col + 1],
                            xs[c][:, t, bass.ts(nh, 512)],
                            start=(col == 0),
                            stop=(col == SC - 1),
                            tile_position=(0, 32 * row),
                        )
            if row == 3:
                for nh in range(H // 512):
                    csb = outp.tile([128, 512], F32, tag="csb")
                    nc.scalar.copy(csb[:], cgrp.pop((gen, nh))[:])
                    for j in range(4):
                        nc.gpsimd.dma_start(
                            out_d[4 * gen + j : 4 * gen + j + 1, bass.ts(nh, 512)],
                            csb[32 * j : 32 * j + 1, :],
                        )

        start_batch(0)
        for c in range(NCH):
            emit_chunk(0, c)
        for b in range(BL):
            if b + 1 < BL:
                start_batch(b + 1)
                emit_chunk(b + 1, 0)
                emit_chunk(b + 1, 1)
            finish_batch(b)
            if b + 1 < BL:
                emit_chunk(b + 1, 2)
                emit_chunk(b + 1, 3)

    nc.compile()
    return nc


_CACHE = {}


def _get_nc():
    if "nc" not in _CACHE:
        _CACHE["nc"] = _build()
    return _CACHE["nc"]


def _make_in_maps(hidden, encoder_output, encoder_mask, W):
    eo = np.ascontiguousarray(encoder_output, dtype=np.float16)
    # W^T with rows permuted [p, c]-major so the device DMA gets 16 KiB
    # contiguous partition lines: wtp[p*HC + c, :] = W.T[c*128 + p, :]
    wt = np.ascontiguousarray(
        np.asarray(W, dtype=np.float16).T
        .reshape(HC, 128, H).transpose(1, 0, 2).reshape(H, H)
    )
    # additive mask in [128p, b, col] layout (col = chunk*RP + t,
    # s = chunk*512 + 4p + t): -1e16 at masked positions
    mk = encoder_mask.reshape(B, S).astype(np.float32) * np.float32(NEG_BIG)
    mk = np.ascontiguousarray(
        mk.reshape(B, NCH, 128, RP).transpose(2, 0, 1, 3).reshape(128, B, SC)
    )
    hid = np.asarray(hidden, dtype=np.float16)
    sel = np.zeros((BL, 128), dtype=np.float16)
    sel[0, :] = 1.0
    in_maps = []
    for i in range(N_CORES):
        sl = slice(i * BL, (i + 1) * BL)
        # ht[p, c*BL + b] = hidden[b, c*128 + p]
        ht = np.ascontiguousarray(
            hid[sl].reshape(BL, HC, 128).transpose(2, 1, 0).reshape(128, HC * BL)
        )
        in_maps.append(
            {
                "hidden": ht,
                "encoder_output": eo[sl],
                "encoder_mask": np.ascontiguousarray(mk[:, sl, :]),
                "W": wt,
                "sel": sel,
            }
        )
    return in_maps


def run(hidden, encoder_output, encoder_mask, W, trace=False):
    nc = _get_nc()
    in_maps = _make_in_maps(hidden, encoder_output, encoder_mask, W)
    res = run_bass_kernel_spmd(nc, in_maps, list(range(N_CORES)), trace=trace)
    out = np.concatenate([res.results[i]["out"] for i in range(N_CORES)], axis=0)
    return out, res


def kernel(hidden, encoder_output, encoder_mask, W):
    out, _ = run(hidden, encoder_output, encoder_mask, W, trace=False)
    return out
